# revision 2
# baseline (speedup 1.0000x reference)
"""Trainium2 Bass kernel v2: row-wise Linear(64->64) + LayerNorm + LeakyReLU(0.2).

Feature-major design: output features stay on PARTITIONS after the GEMM, rows
on the free dim.  The per-row variance reduction and the per-row inv
broadcast both become small mask matmuls on the Tensor engine (DVE cannot
cross partition lanes; bn_stats per tile would bottleneck the DVE).

Per core (data-parallel over 8 cores), banks of 512 columns (1024 rows):
  MM1   psT[(g,o), n] = sum_f Wc[f,o] * x[row(g,n), f]      (PE, wstat lhsT)
  COPY  s = psT + bc  -> SBUF fp16                           (ScalarE Identity,
        per-partition bias AP; frees PSUM immediately)
  SQ    sq = s*s                                             (GpSimd/DVE fp16)
  MM2   var[32c+2b'+g, n] += mask^T @ sq  (M=32, PE)         c = chunk-in-group
        3 chunks (24 banks) pack into one [96,512] PSUM tile at partition
        bases {0,32,64} (PE PSUM writes allow only those bases)
  RSQ   inv = Exp(-0.5*Ln(var/64 + eps))  per 24-bank group  (ScalarE; the
        table-set switch cost amortizes over the group)
  MM3   invb[(g,o), n] = gmask_cb^T @ inv                    (PE broadcast)
  NORM  z = s * invb                                         (DVE TT)
  LRELU z = max(alpha*z, z)   split DVE / ScalarE / GpSimd by column range
Inputs/outputs are fp16 in DRAM (halves HBM traffic; tolerance is 2e-2).

Weights are column-mean-centered on host so psT directly gives y - mean(y).
"""

import numpy as np

import concourse.bass as bass
import concourse.bacc as bacc
import concourse.tile as tile
from concourse import mybir
from concourse.bass_utils import run_bass_kernel_spmd

F32 = mybir.dt.float32
F16 = mybir.dt.float16
I32 = mybir.dt.int32
AX = mybir.AluOpType
AF = mybir.ActivationFunctionType

IN_F = 64
OUT_F = 64
EPS = 1e-5
ALPHA = 0.2
N_CORES = 8
BANK = 512          # fp32 elems per PSUM bank per partition
GROUP_CHUNKS = 3    # chunks per rsqrt group (var slices at bases 0/32/64)

# --- tunables -------------------------------------------------------------
CHUNK_COLS = 4096   # columns per chunk (8 banks)
# leaky-relu split per chunk: fractions of columns -> DVE / ScalarE (rest gpsimd)
LRELU_DVE = 1.0
LRELU_SCE = 0.0
SQ_GP = 1.0         # fraction of square pairs on gpsimd (rest DVE)
IN_BUFS = 3
OUT_BUFS = 3
S_BUFS = 20
SQ_BUFS = 4


def build_module(cols, chunk_cols=None, lrelu_dve=None, lrelu_sce=None,
                 sq_gp=None, dyn_reps=False):
    chunk_cols = CHUNK_COLS if chunk_cols is None else chunk_cols
    lrelu_dve = LRELU_DVE if lrelu_dve is None else lrelu_dve
    lrelu_sce = LRELU_SCE if lrelu_sce is None else lrelu_sce
    sq_gp = SQ_GP if sq_gp is None else sq_gp
    assert cols % BANK == 0
    assert chunk_cols % BANK == 0 and chunk_cols <= 8 * BANK
    nc = bacc.Bacc("TRN2", target_bir_lowering=False, debug=False,
                   enable_asserts=False)
    xh = nc.dram_tensor("xh", [128, cols], F16, kind="ExternalInput").ap()
    if dyn_reps:
        reps = nc.dram_tensor("reps", [1, 1], I32, kind="ExternalInput").ap()
    wstat = nc.dram_tensor("wstat", [128, 128], F16, kind="ExternalInput").ap()
    # MM2 masks: per bank-in-chunk b, [128, 32]; hot cols 2b, 2b+1 select the
    # two groups; remaining cols zero-fill rows so Ln never sees garbage.
    masks2 = nc.dram_tensor("masks2", [128, 8 * 32], F16,
                            kind="ExternalInput").ap()
    # MM3 masks: per (chunk-in-group c, bank b), [96, 128] selecting inv rows
    # 32c+2b+g onto partition blocks g*64..g*64+63.
    gmasks = nc.dram_tensor("gmasks", [96, 24 * 128], F16,
                            kind="ExternalInput").ap()
    bch = nc.dram_tensor("bch", [128, 2], F32, kind="ExternalInput").ap()
    zh = nc.dram_tensor("zh", [128, cols], F16, kind="ExternalOutput").ap()

    chunks = []
    c0 = 0
    while c0 < cols:
        fc = min(chunk_cols, cols - c0)
        chunks.append((c0, fc))
        c0 += fc

    with tile.TileContext(nc) as tc:
        with (
            tc.tile_pool(name="const", bufs=1) as constp,
            tc.tile_pool(name="inp", bufs=IN_BUFS) as inp,
            tc.tile_pool(name="outp", bufs=OUT_BUFS) as outp,
            tc.tile_pool(name="sp", bufs=S_BUFS) as spool,
            tc.tile_pool(name="sqp", bufs=SQ_BUFS) as sqp,
            tc.tile_pool(name="tp", bufs=2) as tpool,
            tc.tile_pool(name="invp", bufs=2) as invp,
            tc.tile_pool(name="ap_", bufs=2, space="PSUM") as apool,
            tc.tile_pool(name="bp_", bufs=2, space="PSUM") as bpool,
            tc.tile_pool(name="cp_", bufs=2, space="PSUM") as cpool,
        ):
            wstat_sb = constp.tile([128, 128], F16, name="wstat_sb")
            nc.sync.dma_start(wstat_sb[:, :], wstat)
            masks2_sb = constp.tile([128, 8, 32], F16, name="masks2_sb")
            nc.sync.dma_start(masks2_sb[:, :, :],
                              masks2.rearrange("p (b m) -> p b m", b=8))
            gmasks_sb = constp.tile([96, 24, 128], F16, name="gmasks_sb")
            nc.sync.dma_start(gmasks_sb[:, :, :],
                              gmasks.rearrange("p (b m) -> p b m", b=24))
            bc_sb = constp.tile([128, 2], F32, name="bc_sb")
            nc.sync.dma_start(bc_sb[:, :], bch)

            import contextlib
            if dyn_reps:
                reps_sb = constp.tile([1, 1], I32, name="reps_sb")
                nc.sync.dma_start(reps_sb[:, :], reps)
                rv = nc.values_load(reps_sb[0:1, 0:1], min_val=0, max_val=64,
                                    skip_runtime_bounds_check=True)
                loop_cm = tc.For_i(0, rv, 1)
            else:
                loop_cm = contextlib.nullcontext()
            with loop_cm:
              gi = 0
              while gi < len(chunks):
                group = chunks[gi:gi + GROUP_CHUNKS]
                gi += GROUP_CHUNKS
                var = cpool.tile([96, BANK], F32, name="var", tag="var")
                inv_sb = invp.tile([96, BANK], F16, name="inv_sb", tag="inv")
                chunk_data = []   # (c0, fc, zout, s_tiles)
                for ci, (c0, fc) in enumerate(group):
                    nbanks = fc // BANK
                    npairs = (nbanks + 1) // 2
                    xin = inp.tile([128, chunk_cols], F16, name="xin",
                                   tag="xin")
                    nc.sync.dma_start(xin[:, 0:fc], xh[:, c0:c0 + fc])
                    zout = outp.tile([128, chunk_cols], F16, name="zout",
                                     tag="zout")
                    s_tiles = []
                    for p in range(npairs):
                        pb = min(2, nbanks - p * 2)
                        a = apool.tile([128, 2 * BANK], F32, name="a", tag="a")
                        for h in range(pb):
                            b = p * 2 + h
                            nc.tensor.matmul(
                                a[:, h * BANK:(h + 1) * BANK],
                                wstat_sb[:, :],
                                xin[:, b * BANK:(b + 1) * BANK],
                                start=True, stop=True, skip_group_check=True,
                            )
                        s_sb = spool.tile([128, 2 * BANK], F16, name="s_sb",
                                          tag="s")
                        s_tiles.append((s_sb, pb))
                        nc.scalar.activation(
                            s_sb[:, 0:pb * BANK], a[:, 0:pb * BANK],
                            AF.Identity, bias=bc_sb[:, 0:1], scale=1.0,
                        )
                        sq = sqp.tile([128, 2 * BANK], F16, name="sq",
                                      tag="sq")
                        eng = (nc.gpsimd if p < sq_gp * npairs else nc.vector)
                        eng.tensor_tensor(
                            sq[:, 0:pb * BANK], s_sb[:, 0:pb * BANK],
                            s_sb[:, 0:pb * BANK], op=AX.mult,
                        )
                        for h in range(pb):
                            b = p * 2 + h
                            nc.tensor.matmul(
                                var[32 * ci:32 * ci + 32, :],
                                masks2_sb[:, b, :],
                                sq[:, h * BANK:(h + 1) * BANK],
                                start=(b == 0), stop=(b == nbanks - 1),
                                skip_group_check=True,
                            )
                    chunk_data.append((c0, fc, zout, s_tiles))
                # rsqrt for the whole group: inv = Exp(-0.5*Ln(var/64 + eps))
                nv = 32 * len(chunk_data)
                nc.scalar.activation(var[0:nv, :], var[0:nv, :], AF.Ln,
                                     bias=bc_sb[0:nv, 1:2], scale=1.0 / 64.0)
                nc.scalar.activation(inv_sb[0:nv, :], var[0:nv, :], AF.Exp,
                                     scale=-0.5)
                # broadcast + normalize + leaky + store per chunk
                for ci, (c0, fc, zout, s_tiles) in enumerate(chunk_data):
                    nbanks = fc // BANK
                    for p in range(len(s_tiles)):
                        s_sb, pb = s_tiles[p]
                        for h in range(pb):
                            b = p * 2 + h
                            invb = bpool.tile([128, BANK], F32, name="invb",
                                              tag="invb")
                            nc.tensor.matmul(
                                invb[:, :],
                                gmasks_sb[0:nv, 8 * ci + b, :],
                                inv_sb[0:nv, :],
                                start=True, stop=True, skip_group_check=True,
                            )
                            nc.vector.tensor_tensor(
                                zout[:, b * BANK:(b + 1) * BANK],
                                s_sb[:, h * BANK:(h + 1) * BANK],
                                invb[:, :], op=AX.mult,
                            )
                    dcols = int(fc * lrelu_dve) // 128 * 128
                    scols = int(fc * lrelu_sce) // 128 * 128
                    if lrelu_dve + lrelu_sce >= 0.999:
                        scols = fc - dcols
                    if lrelu_dve >= 0.999:
                        dcols, scols = fc, 0
                    gcols = fc - dcols - scols
                    ofs = 0
                    if dcols:
                        zc = zout[:, ofs:ofs + dcols]
                        nc.vector.scalar_tensor_tensor(
                            zc, zc, ALPHA, zc, op0=AX.mult, op1=AX.max)
                        ofs += dcols
                    if scols:
                        zc = zout[:, ofs:ofs + scols]
                        nc.scalar.activation(zc, zc, AF.Prelu, alpha=ALPHA)
                        ofs += scols
                    if gcols:
                        # Pool lacks scalar_tensor_tensor: alpha*z on DVE
                        # (tensor_scalar, 4x fp16), max on Pool tensor_tensor.
                        zc = zout[:, ofs:ofs + gcols]
                        t = tpool.tile([128, chunk_cols], F16, name="t",
                                       tag="t")
                        tc_ = t[:, 0:gcols]
                        nc.vector.tensor_scalar(tc_, zc, ALPHA, None,
                                                op0=AX.mult)
                        nc.gpsimd.tensor_tensor(zc, tc_, zc, op=AX.max)
                    nc.sync.dma_start(zh[:, c0:c0 + fc], zout[:, 0:fc])

    nc.compile()
    return nc


# ---------------------------------------------------------------------------
# host-side packing / unpacking
# ---------------------------------------------------------------------------

def _pack_core(shard, cols):
    """[rows, 64] f32 -> xh [128, cols] fp16: xh[g*64+f, n] = x[g*cols+n, f]"""
    rows = shard.shape[0]
    half = (rows + 1) // 2
    xpad = np.zeros((2 * cols, IN_F), dtype=np.float32)
    xpad[:half] = shard[:half]
    xpad[cols:cols + rows - half] = shard[half:]
    xh = xpad.reshape(2, cols, IN_F).transpose(0, 2, 1).reshape(128, cols)
    return np.ascontiguousarray(xh.astype(np.float16))


def _unpack_core(zh, cols, rows):
    """zh [128, cols] fp16 -> [rows, 64] f32: out[g*cols+n, o] = zh[g*64+o, n]"""
    half = (rows + 1) // 2
    zz = zh.astype(np.float32).reshape(2, OUT_F, cols).transpose(0, 2, 1)
    return np.concatenate([zz[0, :half], zz[1, :rows - half]], axis=0)


def _make_weights(W, b):
    Wt = W.astype(np.float64).T                      # [in_f, out_f]
    Wc = Wt - Wt.mean(axis=1, keepdims=True)
    wstat = np.zeros((128, 128), dtype=np.float64)   # lhsT [K=(g,f), M=(g,o)]
    wstat[:64, :64] = Wc
    wstat[64:, 64:] = Wc
    bc = (b.astype(np.float64) - b.astype(np.float64).mean()).astype(np.float32)
    bch = np.zeros((128, 2), dtype=np.float32)
    bch[:, 0] = np.tile(bc, 2)
    bch[:, 1] = EPS
    masks2 = np.zeros((128, 8, 32), dtype=np.float16)  # lhsT [K=(g,o), M=32]
    for bb in range(8):
        for g in range(2):
            masks2[g * 64:(g + 1) * 64, bb, 2 * bb + g] = 1.0
    # MM3 lhsT [K=96, M=(g,o)] per (chunk-in-group c, bank b)
    gmasks = np.zeros((96, 24, 128), dtype=np.float16)
    for cc in range(3):
        for bb in range(8):
            for g in range(2):
                gmasks[32 * cc + 2 * bb + g, 8 * cc + bb,
                       g * 64:(g + 1) * 64] = 1.0
    return (wstat.astype(np.float16), masks2.reshape(128, 256),
            gmasks.reshape(96, 24 * 128), bch)


_NC_CACHE = {}


def _get_module(cols):
    key = (cols, CHUNK_COLS, LRELU_DVE, LRELU_SCE, SQ_GP)
    if key not in _NC_CACHE:
        _NC_CACHE[key] = build_module(cols)
    return _NC_CACHE[key]


def _host_reference(input_x, W, b, gamma, beta):
    y = input_x.astype(np.float32) @ W.T.astype(np.float32) + b
    mu = y.mean(axis=-1, keepdims=True)
    var = np.square(y - mu).mean(axis=-1, keepdims=True)
    y = (y - mu) / np.sqrt(var + EPS) * gamma + beta
    return np.where(y >= 0, y, np.float32(ALPHA) * y).astype(np.float32)


def _make_in_maps(input_x, W, b):
    n = input_x.shape[0]
    per_core = (n + N_CORES - 1) // N_CORES
    per_core += (-per_core) % 2
    half = per_core // 2
    cols = ((half + BANK - 1) // BANK) * BANK
    wstat, masks2, gmasks, bch = _make_weights(W, b)
    in_maps = []
    shards = []
    for i in range(N_CORES):
        lo = min(i * per_core, n)
        hi = min(lo + per_core, n)
        shard = input_x[lo:hi]
        if shard.shape[0] < per_core:
            shard = np.concatenate(
                [shard, np.zeros((per_core - shard.shape[0], IN_F), np.float32)]
            )
        shards.append((lo, hi))
        in_maps.append(
            {"xh": _pack_core(shard, cols), "wstat": wstat, "masks2": masks2,
             "gmasks": gmasks, "bch": bch}
        )
    return in_maps, shards, cols, per_core


def kernel(input_x, W, b, gamma, beta, batch=None, **_unused):
    input_x = np.asarray(input_x, dtype=np.float32)
    W = np.asarray(W, dtype=np.float32)
    b = np.asarray(b, dtype=np.float32)
    gamma = np.asarray(gamma, dtype=np.float32)
    beta = np.asarray(beta, dtype=np.float32)

    if not (np.all(gamma == 1.0) and np.all(beta == 0.0)):
        return _host_reference(input_x, W, b, gamma, beta)

    n = input_x.shape[0]
    in_maps, shards, cols, per_core = _make_in_maps(input_x, W, b)
    nc = _get_module(cols)
    res = run_bass_kernel_spmd(nc, in_maps, core_ids=list(range(N_CORES)))

    out = np.empty((n, OUT_F), dtype=np.float32)
    for i, (lo, hi) in enumerate(shards):
        zh = np.asarray(res.results[i]["zh"])
        z = _unpack_core(zh, cols, per_core)
        out[lo:hi] = z[: hi - lo]
    return out


# revision 3
# speedup vs baseline: 1.0515x; 1.0515x over previous
"""Trainium2 Bass kernel v2: row-wise Linear(64->64) + LayerNorm + LeakyReLU(0.2).

Feature-major design: output features stay on PARTITIONS after the GEMM, rows
on the free dim.  The per-row variance reduction and the per-row inv
broadcast both become small mask matmuls on the Tensor engine (DVE cannot
cross partition lanes; bn_stats per tile would bottleneck the DVE).

Per core (data-parallel over 8 cores), banks of 512 columns (1024 rows):
  MM1   psT[(g,o), n] = sum_f Wc[f,o] * x[row(g,n), f]      (PE, wstat lhsT)
  COPY  s = psT + bc  -> SBUF fp16                           (ScalarE Identity,
        per-partition bias AP; frees PSUM immediately)
  SQ    sq = s*s                                             (GpSimd/DVE fp16)
  MM2   var[32c+2b'+g, n] += mask^T @ sq  (M=32, PE)         c = chunk-in-group
        3 chunks (24 banks) pack into one [96,512] PSUM tile at partition
        bases {0,32,64} (PE PSUM writes allow only those bases)
  RSQ   inv = Exp(-0.5*Ln(var/64 + eps))  per 24-bank group  (ScalarE; the
        table-set switch cost amortizes over the group)
  MM3   invb[(g,o), n] = gmask_cb^T @ inv                    (PE broadcast)
  NORM  z = s * invb                                         (DVE TT)
  LRELU z = max(alpha*z, z)   split DVE / ScalarE / GpSimd by column range
Inputs/outputs are fp16 in DRAM (halves HBM traffic; tolerance is 2e-2).

Weights are column-mean-centered on host so psT directly gives y - mean(y).
"""

import numpy as np

import concourse.bass as bass
import concourse.bacc as bacc
import concourse.tile as tile
from concourse import mybir
from concourse.bass_utils import run_bass_kernel_spmd

F32 = mybir.dt.float32
F16 = mybir.dt.float16
I32 = mybir.dt.int32
AX = mybir.AluOpType
AF = mybir.ActivationFunctionType

IN_F = 64
OUT_F = 64
EPS = 1e-5
ALPHA = 0.2
N_CORES = 8
BANK = 512          # fp32 elems per PSUM bank per partition
GROUP_CHUNKS = 3    # chunks per rsqrt group (var slices at bases 0/32/64)

# --- tunables -------------------------------------------------------------
CHUNK_COLS = 4096   # columns per chunk (8 banks)
# leaky-relu split per chunk: fractions of columns -> DVE / ScalarE (rest gpsimd)
LRELU_DVE = 1.0
LRELU_SCE = 0.0
SQ_GP = 1.0         # fraction of square pairs on gpsimd (rest DVE)
IN_BUFS = 4
OUT_BUFS = 4
S_BUFS = 24
SQ_BUFS = 4
STORE_ENG = "scalar"   # second HWDGE queue for output stores
LRELU_SPLITS = 2       # lrelu ops per chunk (finer overlap with norms)


def build_module(cols, chunk_cols=None, lrelu_dve=None, lrelu_sce=None,
                 sq_gp=None, dyn_reps=False):
    chunk_cols = CHUNK_COLS if chunk_cols is None else chunk_cols
    lrelu_dve = LRELU_DVE if lrelu_dve is None else lrelu_dve
    lrelu_sce = LRELU_SCE if lrelu_sce is None else lrelu_sce
    sq_gp = SQ_GP if sq_gp is None else sq_gp
    assert cols % BANK == 0
    assert chunk_cols % BANK == 0 and chunk_cols <= 8 * BANK
    nc = bacc.Bacc("TRN2", target_bir_lowering=False, debug=False,
                   enable_asserts=False)
    xh = nc.dram_tensor("xh", [128, cols], F16, kind="ExternalInput").ap()
    if dyn_reps:
        reps = nc.dram_tensor("reps", [1, 1], I32, kind="ExternalInput").ap()
    wstat = nc.dram_tensor("wstat", [128, 128], F16, kind="ExternalInput").ap()
    # MM2 masks: per bank-in-chunk b, [128, 32]; hot cols 2b, 2b+1 select the
    # two groups; remaining cols zero-fill rows so Ln never sees garbage.
    masks2 = nc.dram_tensor("masks2", [128, 8 * 32], F16,
                            kind="ExternalInput").ap()
    # MM3 masks: per (chunk-in-group c, bank b), [96, 128] selecting inv rows
    # 32c+2b+g onto partition blocks g*64..g*64+63.
    gmasks = nc.dram_tensor("gmasks", [96, 24 * 128], F16,
                            kind="ExternalInput").ap()
    bch = nc.dram_tensor("bch", [128, 2], F32, kind="ExternalInput").ap()
    zh = nc.dram_tensor("zh", [128, cols], F16, kind="ExternalOutput").ap()

    chunks = []
    c0 = 0
    while c0 < cols:
        fc = min(chunk_cols, cols - c0)
        chunks.append((c0, fc))
        c0 += fc

    with tile.TileContext(nc) as tc:
        with (
            tc.tile_pool(name="const", bufs=1) as constp,
            tc.tile_pool(name="inp", bufs=IN_BUFS) as inp,
            tc.tile_pool(name="outp", bufs=OUT_BUFS) as outp,
            tc.tile_pool(name="sp", bufs=S_BUFS) as spool,
            tc.tile_pool(name="sqp", bufs=SQ_BUFS) as sqp,
            tc.tile_pool(name="tp", bufs=2) as tpool,
            tc.tile_pool(name="invp", bufs=2) as invp,
            tc.tile_pool(name="ap_", bufs=2, space="PSUM") as apool,
            tc.tile_pool(name="bp_", bufs=3, space="PSUM") as bpool,
            tc.tile_pool(name="cp_", bufs=1, space="PSUM") as cpool,
        ):
            wstat_sb = constp.tile([128, 128], F16, name="wstat_sb")
            nc.sync.dma_start(wstat_sb[:, :], wstat)
            masks2_sb = constp.tile([128, 8, 32], F16, name="masks2_sb")
            nc.sync.dma_start(masks2_sb[:, :, :],
                              masks2.rearrange("p (b m) -> p b m", b=8))
            gmasks_sb = constp.tile([96, 24, 128], F16, name="gmasks_sb")
            nc.sync.dma_start(gmasks_sb[:, :, :],
                              gmasks.rearrange("p (b m) -> p b m", b=24))
            bc_sb = constp.tile([128, 2], F32, name="bc_sb")
            nc.sync.dma_start(bc_sb[:, :], bch)

            import contextlib
            if dyn_reps:
                reps_sb = constp.tile([1, 1], I32, name="reps_sb")
                nc.sync.dma_start(reps_sb[:, :], reps)
                rv = nc.values_load(reps_sb[0:1, 0:1], min_val=0, max_val=64,
                                    skip_runtime_bounds_check=True)
                loop_cm = tc.For_i(0, rv, 1)
            else:
                loop_cm = contextlib.nullcontext()
            with loop_cm:
              gi = 0
              while gi < len(chunks):
                group = chunks[gi:gi + GROUP_CHUNKS]
                gi += GROUP_CHUNKS
                var = cpool.tile([96, BANK], F32, name="var", tag="var")
                inv_sb = invp.tile([96, BANK], F16, name="inv_sb", tag="inv")
                chunk_data = []   # (c0, fc, zout, s_tiles)
                for ci, (c0, fc) in enumerate(group):
                    nbanks = fc // BANK
                    npairs = (nbanks + 1) // 2
                    xin = inp.tile([128, chunk_cols], F16, name="xin",
                                   tag="xin")
                    nc.sync.dma_start(xin[:, 0:fc], xh[:, c0:c0 + fc])
                    zout = outp.tile([128, chunk_cols], F16, name="zout",
                                     tag="zout")
                    s_tiles = []
                    for p in range(npairs):
                        pb = min(2, nbanks - p * 2)
                        a = apool.tile([128, 2 * BANK], F32, name="a", tag="a")
                        for h in range(pb):
                            b = p * 2 + h
                            nc.tensor.matmul(
                                a[:, h * BANK:(h + 1) * BANK],
                                wstat_sb[:, :],
                                xin[:, b * BANK:(b + 1) * BANK],
                                start=True, stop=True, skip_group_check=True,
                            )
                        s_sb = spool.tile([128, 2 * BANK], F16, name="s_sb",
                                          tag="s")
                        s_tiles.append((s_sb, pb))
                        nc.scalar.activation(
                            s_sb[:, 0:pb * BANK], a[:, 0:pb * BANK],
                            AF.Identity, bias=bc_sb[:, 0:1], scale=1.0,
                        )
                        sq = sqp.tile([128, 2 * BANK], F16, name="sq",
                                      tag="sq")
                        eng = (nc.gpsimd if p < sq_gp * npairs else nc.vector)
                        eng.tensor_tensor(
                            sq[:, 0:pb * BANK], s_sb[:, 0:pb * BANK],
                            s_sb[:, 0:pb * BANK], op=AX.mult,
                        )
                        for h in range(pb):
                            b = p * 2 + h
                            nc.tensor.matmul(
                                var[32 * ci:32 * ci + 32, :],
                                masks2_sb[:, b, :],
                                sq[:, h * BANK:(h + 1) * BANK],
                                start=(b == 0), stop=(b == nbanks - 1),
                                skip_group_check=True,
                            )
                    chunk_data.append((c0, fc, zout, s_tiles))
                # rsqrt for the whole group: inv = Exp(-0.5*Ln(var/64 + eps))
                nv = 32 * len(chunk_data)
                nc.scalar.activation(var[0:nv, :], var[0:nv, :], AF.Ln,
                                     bias=bc_sb[0:nv, 1:2], scale=1.0 / 64.0)
                nc.scalar.activation(inv_sb[0:nv, :], var[0:nv, :], AF.Exp,
                                     scale=-0.5)
                # broadcast + normalize + leaky + store per chunk
                for ci, (c0, fc, zout, s_tiles) in enumerate(chunk_data):
                    nbanks = fc // BANK
                    for p in range(len(s_tiles)):
                        s_sb, pb = s_tiles[p]
                        for h in range(pb):
                            b = p * 2 + h
                            invb = bpool.tile([128, BANK], F32, name="invb",
                                              tag="invb")
                            nc.tensor.matmul(
                                invb[:, :],
                                gmasks_sb[0:nv, 8 * ci + b, :],
                                inv_sb[0:nv, :],
                                start=True, stop=True, skip_group_check=True,
                            )
                            nc.vector.tensor_tensor(
                                zout[:, b * BANK:(b + 1) * BANK],
                                s_sb[:, h * BANK:(h + 1) * BANK],
                                invb[:, :], op=AX.mult,
                            )
                    dcols = int(fc * lrelu_dve) // 128 * 128
                    scols = int(fc * lrelu_sce) // 128 * 128
                    if lrelu_dve + lrelu_sce >= 0.999:
                        scols = fc - dcols
                    if lrelu_dve >= 0.999:
                        dcols, scols = fc, 0
                    gcols = fc - dcols - scols
                    ofs = 0
                    if dcols:
                        # split into pieces so each lrelu op only waits on
                        # the norms covering its columns
                        step = max(BANK, (dcols // LRELU_SPLITS) // BANK * BANK)
                        o2 = 0
                        while o2 < dcols:
                            w = min(step, dcols - o2)
                            zc = zout[:, ofs + o2:ofs + o2 + w]
                            nc.vector.scalar_tensor_tensor(
                                zc, zc, ALPHA, zc, op0=AX.mult, op1=AX.max)
                            o2 += w
                        ofs += dcols
                    if scols:
                        zc = zout[:, ofs:ofs + scols]
                        nc.scalar.activation(zc, zc, AF.Prelu, alpha=ALPHA)
                        ofs += scols
                    if gcols:
                        # Pool lacks scalar_tensor_tensor: alpha*z on DVE
                        # (tensor_scalar, 4x fp16), max on Pool tensor_tensor.
                        zc = zout[:, ofs:ofs + gcols]
                        t = tpool.tile([128, chunk_cols], F16, name="t",
                                       tag="t")
                        tc_ = t[:, 0:gcols]
                        nc.vector.tensor_scalar(tc_, zc, ALPHA, None,
                                                op0=AX.mult)
                        nc.gpsimd.tensor_tensor(zc, tc_, zc, op=AX.max)
                    getattr(nc, STORE_ENG).dma_start(zh[:, c0:c0 + fc],
                                                     zout[:, 0:fc])

    nc.compile()
    return nc


# ---------------------------------------------------------------------------
# host-side packing / unpacking
# ---------------------------------------------------------------------------

def _pack_core(shard, cols):
    """[rows, 64] f32 -> xh [128, cols] fp16: xh[g*64+f, n] = x[g*cols+n, f]"""
    rows = shard.shape[0]
    half = (rows + 1) // 2
    xpad = np.zeros((2 * cols, IN_F), dtype=np.float32)
    xpad[:half] = shard[:half]
    xpad[cols:cols + rows - half] = shard[half:]
    xh = xpad.reshape(2, cols, IN_F).transpose(0, 2, 1).reshape(128, cols)
    return np.ascontiguousarray(xh.astype(np.float16))


def _unpack_core(zh, cols, rows):
    """zh [128, cols] fp16 -> [rows, 64] f32: out[g*cols+n, o] = zh[g*64+o, n]"""
    half = (rows + 1) // 2
    zz = zh.astype(np.float32).reshape(2, OUT_F, cols).transpose(0, 2, 1)
    return np.concatenate([zz[0, :half], zz[1, :rows - half]], axis=0)


def _make_weights(W, b):
    Wt = W.astype(np.float64).T                      # [in_f, out_f]
    Wc = Wt - Wt.mean(axis=1, keepdims=True)
    wstat = np.zeros((128, 128), dtype=np.float64)   # lhsT [K=(g,f), M=(g,o)]
    wstat[:64, :64] = Wc
    wstat[64:, 64:] = Wc
    bc = (b.astype(np.float64) - b.astype(np.float64).mean()).astype(np.float32)
    bch = np.zeros((128, 2), dtype=np.float32)
    bch[:, 0] = np.tile(bc, 2)
    bch[:, 1] = EPS
    masks2 = np.zeros((128, 8, 32), dtype=np.float16)  # lhsT [K=(g,o), M=32]
    for bb in range(8):
        for g in range(2):
            masks2[g * 64:(g + 1) * 64, bb, 2 * bb + g] = 1.0
    # MM3 lhsT [K=96, M=(g,o)] per (chunk-in-group c, bank b)
    gmasks = np.zeros((96, 24, 128), dtype=np.float16)
    for cc in range(3):
        for bb in range(8):
            for g in range(2):
                gmasks[32 * cc + 2 * bb + g, 8 * cc + bb,
                       g * 64:(g + 1) * 64] = 1.0
    return (wstat.astype(np.float16), masks2.reshape(128, 256),
            gmasks.reshape(96, 24 * 128), bch)


_NC_CACHE = {}


def _get_module(cols):
    key = (cols, CHUNK_COLS, LRELU_DVE, LRELU_SCE, SQ_GP)
    if key not in _NC_CACHE:
        _NC_CACHE[key] = build_module(cols)
    return _NC_CACHE[key]


def _host_reference(input_x, W, b, gamma, beta):
    y = input_x.astype(np.float32) @ W.T.astype(np.float32) + b
    mu = y.mean(axis=-1, keepdims=True)
    var = np.square(y - mu).mean(axis=-1, keepdims=True)
    y = (y - mu) / np.sqrt(var + EPS) * gamma + beta
    return np.where(y >= 0, y, np.float32(ALPHA) * y).astype(np.float32)


def _make_in_maps(input_x, W, b):
    n = input_x.shape[0]
    per_core = (n + N_CORES - 1) // N_CORES
    per_core += (-per_core) % 2
    half = per_core // 2
    cols = ((half + BANK - 1) // BANK) * BANK
    wstat, masks2, gmasks, bch = _make_weights(W, b)
    in_maps = []
    shards = []
    for i in range(N_CORES):
        lo = min(i * per_core, n)
        hi = min(lo + per_core, n)
        shard = input_x[lo:hi]
        if shard.shape[0] < per_core:
            shard = np.concatenate(
                [shard, np.zeros((per_core - shard.shape[0], IN_F), np.float32)]
            )
        shards.append((lo, hi))
        in_maps.append(
            {"xh": _pack_core(shard, cols), "wstat": wstat, "masks2": masks2,
             "gmasks": gmasks, "bch": bch}
        )
    return in_maps, shards, cols, per_core


def kernel(input_x, W, b, gamma, beta, batch=None, **_unused):
    input_x = np.asarray(input_x, dtype=np.float32)
    W = np.asarray(W, dtype=np.float32)
    b = np.asarray(b, dtype=np.float32)
    gamma = np.asarray(gamma, dtype=np.float32)
    beta = np.asarray(beta, dtype=np.float32)

    if not (np.all(gamma == 1.0) and np.all(beta == 0.0)):
        return _host_reference(input_x, W, b, gamma, beta)

    n = input_x.shape[0]
    in_maps, shards, cols, per_core = _make_in_maps(input_x, W, b)
    nc = _get_module(cols)
    res = run_bass_kernel_spmd(nc, in_maps, core_ids=list(range(N_CORES)))

    out = np.empty((n, OUT_F), dtype=np.float32)
    for i, (lo, hi) in enumerate(shards):
        zh = np.asarray(res.results[i]["zh"])
        z = _unpack_core(zh, cols, per_core)
        out[lo:hi] = z[: hi - lo]
    return out
